# revision 1
# baseline (speedup 1.0000x reference)
"""Trainium2 Bass kernel for nn_Correlation (stereo cost volume).

  out[b, d, h, w] = mean_c( x[b,c,h,w] * y[b,c,h,w-d] ),  w >= d else 0
  B=8, C=32, H=256, W=512, D=48  (maxdisp=48)

Sharding: data-parallel over batch B across the 8 NeuronCores (one batch
element per core).  Each core computes its full [D, H, W] cost volume.

Per-core algorithm (fp32 compute, fp16 staging/output storage):
  - x/y rows are staged in SBUF in two 32-partition slabs (partitions
    0-31 and 64-95) so consecutive matmuls alternate PE row groups and
    LDWEIGHTS overlaps the running matmul.  y rows are stored
    contiguously with a 47-col lead so a single DMA per slab loads all
    G rows (windows that read across row boundaries only feed the w<d
    zone, which is zeroed later).
  - Per (h, 128-col w-tile): one PE matmul, K=C=32, stationary = X
    columns [32,128], moving = Y window [32,175].  psum[j, u] =
    <x_col(w0+j), y_col(w0+u-47)>, so the 48 outputs of column j sit on
    the diagonal u = j..j+47 (d = j+47-u).
  - DVE/ACT copies scale psum by 1/32 (the exact mean) into an SBUF
    G tile stored as fp16 (halves all downstream traffic; ~5e-4 rel
    error, values are O(1) means of N(0,1) products so no overflow);
    the w<d zone (cols 0:47 of w-tile 0) is zeroed.
  - G is dumped contiguously to a DRAM scratch, and a skewed
    DRAM->DRAM DMA (DRAM is linear, so arbitrary strides are legal -
    SBUF-side skewed access patterns mis-lower in the DGE descriptor
    generation, resetting the per-partition byte skew every 4
    partitions) walks the band diagonals straight into the output in
    [h, w, d_rev] layout with fully contiguous 98KB writes per row.
  - The host casts back to fp32, reverses d and transposes to
    [d, h, w].
"""

import sys

sys.path.insert(0, "/opt/trn_rl_repo")

import numpy as np
from contextlib import ExitStack

import concourse.bass as bass
import concourse.tile as tile
from concourse import mybir
from concourse import bass_utils

B = 8
C = 32
H = 256
W = 512
D = 48
NW = W // 128           # 4 w-tiles per row
MMN = 128 + D - 1       # 175 moving columns per matmul
LEAD = D - 1            # 47
GW = NW * MMN           # 700 G cols per h
G = 16                  # h rows per slab per iteration


def _split_waits(nc, max_waits=1):
    """Walrus codegen accepts at most ONE sync wait per instruction; Tile
    attaches several.  Split extra waits onto preceding NoOps on the same
    engine queue (dispatch is in-order, waits gate dispatch)."""
    for fn in nc.m.functions:
        for blk in fn.blocks:
            newl = []
            changed = False
            for inst in blk.instructions:
                si = getattr(inst, "sync_info", None)
                ow = list(si.on_wait) if si is not None and si.on_wait else []
                if len(ow) > max_waits and inst.engine is not None:
                    for k, wcond in enumerate(ow[:-max_waits]):
                        newl.append(mybir.InstNoOp(
                            name=f"{inst.name}w{k}",
                            engine=inst.engine,
                            sync_info=mybir.SyncInfo(on_wait=[wcond],
                                                     on_update=[]),
                        ))
                    inst.sync_info = mybir.SyncInfo(
                        on_wait=ow[-max_waits:],
                        on_update=list(si.on_update) if si.on_update else [])
                    changed = True
                newl.append(inst)
            if changed:
                blk.instructions = newl


def _emit_body(ctx, tc, x_ap, y_ap, o_ap, act_frac=0.34):
    nc = tc.nc
    n_iter = H // (2 * G)
    o_t = o_ap.tensor
    yflat = y_ap.rearrange("c h w -> c (h w)")

    # DRAM scratch: one [128, 2*GW] block per h-pair
    gd = nc.dram_tensor("gd", [(H // 2) * 128 * 2 * GW], mybir.dt.float16,
                        kind="Internal")

    xpool = ctx.enter_context(tc.tile_pool(name="xp", bufs=2))
    ypool = ctx.enter_context(tc.tile_pool(name="yp", bufs=2))
    gpool = ctx.enter_context(tc.tile_pool(name="gp", bufs=3))
    ppool = ctx.enter_context(tc.tile_pool(name="pp", bufs=6, space="PSUM"))

    inv_c = 1.0 / C
    hcount = 0

    for it in range(n_iter):
        h0 = it * 2 * G
        xt = xpool.tile([128, G * W], mybir.dt.float32, name=f"xt{it}", tag="xt")
        yt = ypool.tile([128, LEAD + G * W], mybir.dt.float32,
                        name=f"yt{it}", tag="yt")

        nc.sync.dma_start(xt[0:C, :], x_ap[:, h0:h0 + G, :])
        nc.sync.dma_start(xt[64:64 + C, :], x_ap[:, h0 + G:h0 + 2 * G, :])
        if it == 0:
            # no rows before row 0: lead cols stay unloaded; the very first
            # w-tile uses a shrunk moving window instead
            nc.sync.dma_start(yt[0:C, LEAD:], yflat[:, 0:G * W])
        else:
            nc.sync.dma_start(yt[0:C, :], yflat[:, h0 * W - LEAD:(h0 + G) * W])
        nc.sync.dma_start(yt[64:64 + C, :],
                          yflat[:, (h0 + G) * W - LEAD:(h0 + 2 * G) * W])

        for g in range(G):
            hs = (h0 + g, h0 + G + g)
            bases = (0, 64)
            gt = gpool.tile([128, 2 * GW], mybir.dt.float16,
                            name=f"gt{it}_{g}", tag="gt")
            psums = []
            for half in range(NW // 2):           # psum pair = 2 w-tiles
                ps = [
                    ppool.tile([128, 2 * MMN], mybir.dt.float32,
                               name=f"ps{it}_{g}_{half}_{s}", tag="ps",
                               padded_shape=[128, 512])
                    for s in range(2)
                ]
                for wsub in range(2):
                    wt = half * 2 + wsub
                    for s in range(2):
                        base = bases[s]
                        lhs = xt[base:base + C,
                                 g * W + wt * 128: g * W + wt * 128 + 128]
                        lo = LEAD if (it == 0 and g == 0 and s == 0
                                      and wt == 0) else 0
                        rhs = yt[base:base + C,
                                 g * W + wt * 128 + lo: g * W + wt * 128 + MMN]
                        nc.tensor.matmul(
                            ps[s][:, wsub * MMN + lo:(wsub + 1) * MMN],
                            lhs, rhs, start=True, stop=True)
                psums.append(ps)

            for s in range(2):
                for half in range(NW // 2):
                    lo = LEAD if (it == 0 and g == 0 and s == 0
                                  and half == 0) else 0
                    dst_sl = gt[:, s * GW + half * 2 * MMN + lo:
                                s * GW + (half + 1) * 2 * MMN]
                    src_sl = psums[half][s][:, lo:]
                    if (hcount % 100) < act_frac * 100:
                        nc.scalar.mul(dst_sl, src_sl, inv_c)
                    else:
                        nc.vector.tensor_scalar_mul(dst_sl, src_sl, inv_c)
                # zero the w<d zone (read from left of the row start)
                nc.vector.memset(gt[:, s * GW:s * GW + LEAD], 0.0)
                hcount += 1

            # dump the h-pair G to DRAM scratch (contiguous 717KB)
            pc = it * G + g
            dmp = bass.AP(gd, pc * 128 * 2 * GW, [[2 * GW, 128], [1, 2 * GW]])
            nc.sync.dma_start(dmp, gt[:, :])
            # skewed extraction per h: band diagonals -> [h, w, d_rev]
            # (all DMAs stay on the SP HWDGE ring: moving any to the ACT
            # ring serializes with the scalar-engine psum drains and
            # measured 27% slower)
            for s in range(2):
                h = hs[s]
                src = bass.AP(gd, pc * 128 * 2 * GW + s * GW,
                              [[2 * GW + 1, 128], [MMN, NW], [1, D]])
                dst = bass.AP(o_t, h * W * D,
                              [[D, 128], [128 * D, NW], [1, D]])
                nc.sync.dma_start(dst, src)


def _build_kernel():
    nc = bass.Bass(trn_type="TRN2", target_bir_lowering=False)
    x_d = nc.dram_tensor("x", [C, H, W], mybir.dt.float32, kind="ExternalInput")
    y_d = nc.dram_tensor("y", [C, H, W], mybir.dt.float32, kind="ExternalInput")
    o_d = nc.dram_tensor("o", [H, W, D], mybir.dt.float16,
                          kind="ExternalOutput")
    with ExitStack() as ctx:
        tc = ctx.enter_context(tile.TileContext(nc))
        _emit_body(ctx, tc, x_d.ap(), y_d.ap(), o_d.ap())
    _split_waits(nc)
    return nc


_NC_CACHE = None


def _get_nc():
    global _NC_CACHE
    if _NC_CACHE is None:
        _NC_CACHE = _build_kernel()
    return _NC_CACHE


def kernel(x: np.ndarray, y: np.ndarray, maxdisp=48) -> np.ndarray:
    assert int(maxdisp) == D
    x = np.ascontiguousarray(np.asarray(x, dtype=np.float32))
    y = np.ascontiguousarray(np.asarray(y, dtype=np.float32))
    assert x.shape == (B, C, H, W) and y.shape == (B, C, H, W)

    nc = _get_nc()
    in_maps = [{"x": x[b], "y": y[b]} for b in range(B)]
    res = bass_utils.run_bass_kernel_spmd(nc, in_maps, core_ids=list(range(B)))

    out = np.empty((B, D, H, W), dtype=np.float32)
    for b in range(B):
        ob = np.asarray(res.results[b]["o"], dtype=np.float32)
        out[b] = ob[:, :, ::-1].transpose(2, 0, 1)   # undo d reversal
    return out


if __name__ == "__main__":
    rng = np.random.default_rng(0)
    x = rng.standard_normal((B, C, H, W), dtype=np.float32)
    y = rng.standard_normal((B, C, H, W), dtype=np.float32)
    out = kernel(x=x, y=y, maxdisp=D)
    print("kernel output:", out.shape, out.dtype)



# revision 2
# speedup vs baseline: 1.7524x; 1.7524x over previous
"""Trainium2 Bass kernel for nn_Correlation (stereo cost volume).

  out[b, d, h, w] = mean_c( x[b,c,h,w] * y[b,c,h,w-d] ),  w >= d else 0
  B=8, C=32, H=256, W=512, D=48  (maxdisp=48)

Sharding: data-parallel over batch B across the 8 NeuronCores (one batch
element per core).  Each core computes its full [D, H, W] cost volume.

Per-core algorithm (bf16 matmul inputs, fp32 psum, fp16 staging/output):
  - x/y rows are loaded with a casting SWDGE DMA (f32 HBM -> bf16 SBUF)
    in two 32-partition slabs (partitions 0-31 and 64-95) so consecutive
    matmuls alternate PE row groups and LDWEIGHTS overlaps the running
    matmul.  bf16 runs the PE at 1 cycle/moving-col vs ~4 for fp32.
  - Per (h, 128-col w-tile): one PE matmul, K=C=32, stationary = X
    columns [32,128], moving = Y window [32,175].  psum[j, u] =
    <x_col(w0+j), y_col(w0+u-47)>, so the 48 outputs of column j sit on
    the diagonal u = j..j+47 (reversed d).
  - DVE/ACT drains scale psum by 1/32 into an fp16 G8 tile [128, 5600]
    that interleaves EIGHT h-rows element-wise: col = u*8 + hsub.  This
    is the key layout trick: every band diagonal becomes a contiguous
    (48 d) x (8 h) = 768-byte run, so the deskew gather DMA moves 8x
    fewer, 8x larger packets than a per-h band would (the per-h variant
    measured 44 ns per 96 B packet -> DMA-packet-bound at 589 us).
  - G8 is dumped contiguously to a DRAM scratch block, and a skewed
    DRAM->DRAM DMA (DRAM is linear, so arbitrary strides are legal -
    SBUF-side skewed access patterns mis-lower in the DGE descriptor
    generation) walks the band diagonals straight into the output:
    src runs of 768 B at [j*(GW8+8) + wt*1400], dst fully contiguous
    393 KB per h-block in [hblk, j, wt, (d_rev, hsub)] layout.
  - The w<d zone (cols u<47 of w-tile 0) is never drained; a single
    memset zeroes it per tile (disjoint bytes from the drains).
  - The host casts back to fp32 and unpermutes [hblk,j,wt,d_rev,hsub]
    -> [d, h, w].
"""

import sys

sys.path.insert(0, "/opt/trn_rl_repo")

import numpy as np
from contextlib import ExitStack

import concourse.bass as bass
import concourse.tile as tile
from concourse import mybir
from concourse import bass_utils

B = 8
C = 32
H = 256
W = 512
D = 48
NW = W // 128           # 4 w-tiles per row
MMN = 128 + D - 1       # 175 moving columns per matmul
LEAD = D - 1            # 47
HB = 8                  # h rows interleaved per scratch block
NBLK = H // HB          # 32 h-blocks
GW8 = NW * MMN * HB     # 5600 G8 cols (u * 8 + hsub)
BLKSZ = 128 * GW8       # scratch elements per h-block
DH = D * HB             # 384 contiguous elements per deskew run (768 B)


def _split_waits(nc, max_waits=1):
    """Walrus codegen accepts at most ONE sync wait per instruction; Tile
    attaches several.  Split extra waits onto preceding NoOps on the same
    engine queue (dispatch is in-order, waits gate dispatch)."""
    for fn in nc.m.functions:
        for blk in fn.blocks:
            newl = []
            changed = False
            for inst in blk.instructions:
                si = getattr(inst, "sync_info", None)
                ow = list(si.on_wait) if si is not None and si.on_wait else []
                if len(ow) > max_waits and inst.engine is not None:
                    for k, wcond in enumerate(ow[:-max_waits]):
                        newl.append(mybir.InstNoOp(
                            name=f"{inst.name}w{k}",
                            engine=inst.engine,
                            sync_info=mybir.SyncInfo(on_wait=[wcond],
                                                     on_update=[]),
                        ))
                    inst.sync_info = mybir.SyncInfo(
                        on_wait=ow[-max_waits:],
                        on_update=list(si.on_update) if si.on_update else [])
                    changed = True
                newl.append(inst)
            if changed:
                blk.instructions = newl


def _emit_body(ctx, tc, x_ap, y_ap, o_ap, act_frac=0.46):
    nc = tc.nc
    o_t = o_ap.tensor
    yflat = y_ap.rearrange("c h w -> c (h w)")

    # DRAM scratch: one [128, GW8] fp16 block per h-block of 8 rows
    gd = nc.dram_tensor("gd", [NBLK * BLKSZ], mybir.dt.float16,
                        kind="Internal")

    xpool = ctx.enter_context(tc.tile_pool(name="xp", bufs=2))
    ypool = ctx.enter_context(tc.tile_pool(name="yp", bufs=2))
    gpool = ctx.enter_context(tc.tile_pool(name="gp", bufs=4))
    ppool = ctx.enter_context(tc.tile_pool(name="pp", bufs=6, space="PSUM"))

    inv_c = 1.0 / C
    dcount = 0
    n_iter = H // (2 * HB)          # 16 iterations, 2 h-blocks each

    for it in range(n_iter):
        h0 = it * 2 * HB            # slab A rows [h0, h0+8), B [h0+8, h0+16)
        pcs = (2 * it, 2 * it + 1)
        bases = (0, 64)

        xt = xpool.tile([128, HB * W], mybir.dt.bfloat16,
                        name=f"xt{it}", tag="xt")
        yt = ypool.tile([128, LEAD + HB * W], mybir.dt.bfloat16,
                        name=f"yt{it}", tag="yt")

        # casting loads (SWDGE): f32 HBM -> bf16 SBUF
        nc.gpsimd.dma_start(xt[0:C, :], x_ap[:, h0:h0 + HB, :])
        nc.gpsimd.dma_start(xt[64:64 + C, :], x_ap[:, h0 + HB:h0 + 2 * HB, :])
        if it == 0:
            # no rows before row 0: lead cols stay unloaded; the very first
            # w-tile uses a shrunk moving window instead
            nc.gpsimd.dma_start(yt[0:C, LEAD:], yflat[:, 0:HB * W])
        else:
            nc.gpsimd.dma_start(yt[0:C, :],
                                yflat[:, h0 * W - LEAD:(h0 + HB) * W])
        nc.gpsimd.dma_start(yt[64:64 + C, :],
                            yflat[:, (h0 + HB) * W - LEAD:(h0 + 2 * HB) * W])

        gts = [gpool.tile([128, GW8], mybir.dt.float16,
                          name=f"gt{it}_{s}", tag="gt") for s in range(2)]
        for s in range(2):
            # zero the w<d zone (u<47, all hsub); drains never touch it
            nc.vector.memset(gts[s][:, 0:LEAD * HB], 0.0)

        for g in range(HB):
            psums = []
            for half in range(NW // 2):           # psum pair = 2 w-tiles
                ps = [
                    ppool.tile([128, 2 * MMN], mybir.dt.float32,
                               name=f"ps{it}_{g}_{half}_{s}", tag="ps",
                               padded_shape=[128, 512])
                    for s in range(2)
                ]
                for wsub in range(2):
                    wt = half * 2 + wsub
                    for s in range(2):
                        base = bases[s]
                        lhs = xt[base:base + C,
                                 g * W + wt * 128: g * W + wt * 128 + 128]
                        lo = LEAD if (it == 0 and g == 0 and s == 0
                                      and wt == 0) else 0
                        rhs = yt[base:base + C,
                                 g * W + wt * 128 + lo: g * W + wt * 128 + MMN]
                        nc.tensor.matmul(
                            ps[s][:, wsub * MMN + lo:(wsub + 1) * MMN],
                            lhs, rhs, start=True, stop=True)
                psums.append(ps)

            for s in range(2):
                gt = gts[s]
                for half in range(NW // 2):
                    # half 0 skips v'<47 (= w-tile 0, u<47): always the
                    # zeroed w<d zone
                    lo = LEAD if half == 0 else 0
                    dst_sl = gt[:, (half * 2 * MMN + lo) * HB + g:
                                (half + 1) * 2 * MMN * HB: HB]
                    src_sl = psums[half][s][:, lo:]
                    if (dcount % 13) < act_frac * 13:
                        nc.scalar.mul(dst_sl, src_sl, inv_c)
                    else:
                        nc.vector.tensor_scalar_mul(dst_sl, src_sl, inv_c)
                    dcount += 1

        for s in range(2):
            pc = pcs[s]
            # dump the h-block G8 to DRAM scratch (contiguous 1.43 MB)
            dmp = bass.AP(gd, pc * BLKSZ, [[GW8, 128], [1, GW8]])
            nc.sync.dma_start(dmp, gts[s][:, :])
            # skewed extraction: 768 B diagonal runs -> contiguous output
            src = bass.AP(gd, pc * BLKSZ,
                          [[GW8 + HB, 128], [MMN * HB, NW], [1, DH]])
            dst = bass.AP(o_t, pc * (128 * NW * DH),
                          [[NW * DH, 128], [DH, NW], [1, DH]])
            nc.sync.dma_start(dst, src)


def _build_kernel():
    nc = bass.Bass(trn_type="TRN2", target_bir_lowering=False)
    x_d = nc.dram_tensor("x", [C, H, W], mybir.dt.float32, kind="ExternalInput")
    y_d = nc.dram_tensor("y", [C, H, W], mybir.dt.float32, kind="ExternalInput")
    o_d = nc.dram_tensor("o", [NBLK, 128, NW, DH], mybir.dt.float16,
                         kind="ExternalOutput")
    with ExitStack() as ctx:
        tc = ctx.enter_context(tile.TileContext(nc))
        _emit_body(ctx, tc, x_d.ap(), y_d.ap(), o_d.ap())
    _split_waits(nc)
    return nc


_NC_CACHE = None


def _get_nc():
    global _NC_CACHE
    if _NC_CACHE is None:
        _NC_CACHE = _build_kernel()
    return _NC_CACHE


def kernel(x: np.ndarray, y: np.ndarray, maxdisp=48) -> np.ndarray:
    assert int(maxdisp) == D
    x = np.ascontiguousarray(np.asarray(x, dtype=np.float32))
    y = np.ascontiguousarray(np.asarray(y, dtype=np.float32))
    assert x.shape == (B, C, H, W) and y.shape == (B, C, H, W)

    nc = _get_nc()
    in_maps = [{"x": x[b], "y": y[b]} for b in range(B)]
    res = bass_utils.run_bass_kernel_spmd(nc, in_maps, core_ids=list(range(B)))

    out = np.empty((B, D, H, W), dtype=np.float32)
    for b in range(B):
        ob = np.asarray(res.results[b]["o"])      # [NBLK, 128, NW, DH] fp16
        arr = ob.reshape(NBLK, 128, NW, D, HB)    # [hblk, j, wt, d_rev, hsub]
        out[b] = (arr[:, :, :, ::-1, :]
                  .transpose(3, 0, 4, 2, 1)       # [d, hblk, hsub, wt, j]
                  .reshape(D, H, W)
                  .astype(np.float32))
    return out


if __name__ == "__main__":
    rng = np.random.default_rng(0)
    x = rng.standard_normal((B, C, H, W), dtype=np.float32)
    y = rng.standard_normal((B, C, H, W), dtype=np.float32)
    out = kernel(x=x, y=y, maxdisp=D)
    print("kernel output:", out.shape, out.dtype)


# revision 7
# speedup vs baseline: 1.8337x; 1.0464x over previous
"""Trainium2 Bass kernel for nn_Correlation (stereo cost volume).

  out[b, d, h, w] = mean_c( x[b,c,h,w] * y[b,c,h,w-d] ),  w >= d else 0
  B=8, C=32, H=256, W=512, D=48  (maxdisp=48)

Sharding: data-parallel over batch B across the 8 NeuronCores (one batch
element per core).  Each core computes its full [D, H, W] cost volume.

Per-core algorithm (bf16 matmul inputs, fp32 psum, fp16 output):
  - x/y rows are loaded with a casting SWDGE DMA (f32 HBM -> bf16 SBUF)
    in two 32-partition slabs (partitions 0-31 and 64-95) so consecutive
    matmuls alternate PE row groups and LDWEIGHTS overlaps the running
    matmul.  bf16 runs the PE ~2x faster than fp32.
  - Per (h, 128-col w-tile): one PE matmul, K=C=32, stationary = X
    columns [32,128], moving = Y window [32,175].  psum[j, u] =
    <x_col(w0+j), y_col(w0+u-47)>, so the 48 outputs of column j sit on
    the diagonal u = j..j+47 (reversed d).
  - Pairs of h-rows share one 2-bank psum tile (g at cols [0:350),
    g+1 at [512:862)); a single DVE/ACT drain scales both by 1/32 and
    writes them into an fp16 G8 tile [128, 5600] that interleaves
    EIGHT h-rows element-wise (col = u*8 + hsub).  Writing h-PAIRS as
    4-byte granules (stride 16 B) costs ~1.3 cyc/col vs ~5 for single
    2-byte strided writes - the drains were the v2 bottleneck.
  - The w<d zone (cols u<47 of w-tile 0) is never drained; a single
    memset zeroes it per tile (disjoint bytes from the drains).
  - Each h-block's G8 tile is dumped contiguously to a DRAM scratch
    block, and a skewed DRAM->DRAM DMA walks the 48d x 8h diagonal
    runs (768 B contiguous per (j,wt)) straight into the output,
    fully contiguous 393 KB per block in [hblk, j, wt, (d_rev,hsub)]
    layout.  The DRAM bounce is forced: SBUF-side per-partition byte
    skew only lowers correctly for a single 4-partition group at
    partition 0 (HW resets the skew every 4 partitions; walrus
    codegen cannot encode byte-carrying partition steps in outer AP
    dims at all), so a direct skewed SBUF->DRAM dump is impossible.
    The 8-h interleave is what makes the gather's segments 768 B
    instead of 96 B - the per-h variant was DMA-packet-bound.
  - The host casts back to fp32 and unpermutes [hblk,j,wt,d_rev,hsub]
    -> [d, h, w].
"""

import sys

sys.path.insert(0, "/opt/trn_rl_repo")

import numpy as np
from contextlib import ExitStack

import concourse.bass as bass
import concourse.tile as tile
from concourse import mybir
from concourse import bass_utils

B = 8
C = 32
H = 256
W = 512
D = 48
NW = W // 128           # 4 w-tiles per row
MMN = 128 + D - 1       # 175 moving columns per matmul
LEAD = D - 1            # 47
HB = 8                  # h rows interleaved per output block
NBLK = H // HB          # 32 h-blocks
GW8 = NW * MMN * HB     # 5600 G8 cols (u * 8 + hsub)
DH = D * HB             # 384 contiguous elements per diagonal run (768 B)


def _split_waits(nc, max_waits=1):
    """Walrus codegen accepts at most ONE sync wait per instruction; Tile
    attaches several.  Split extra waits onto preceding NoOps on the same
    engine queue (dispatch is in-order, waits gate dispatch)."""
    for fn in nc.m.functions:
        for blk in fn.blocks:
            newl = []
            changed = False
            for inst in blk.instructions:
                si = getattr(inst, "sync_info", None)
                ow = list(si.on_wait) if si is not None and si.on_wait else []
                if len(ow) > max_waits and inst.engine is not None:
                    for k, wcond in enumerate(ow[:-max_waits]):
                        newl.append(mybir.InstNoOp(
                            name=f"{inst.name}w{k}",
                            engine=inst.engine,
                            sync_info=mybir.SyncInfo(on_wait=[wcond],
                                                     on_update=[]),
                        ))
                    inst.sync_info = mybir.SyncInfo(
                        on_wait=ow[-max_waits:],
                        on_update=list(si.on_update) if si.on_update else [])
                    changed = True
                newl.append(inst)
            if changed:
                blk.instructions = newl


def _emit_body(ctx, tc, x_ap, y_ap, o_ap, act_frac=0.5):
    nc = tc.nc
    o_t = o_ap.tensor
    yflat = y_ap.rearrange("c h w -> c (h w)")

    # DRAM scratch: one [128, GW8] fp16 block per h-block of 8 rows
    gd = nc.dram_tensor("gd", [NBLK * 128 * GW8], mybir.dt.float16,
                        kind="Internal")

    xpool = ctx.enter_context(tc.tile_pool(name="xp", bufs=2))
    ypool = ctx.enter_context(tc.tile_pool(name="yp", bufs=2))
    gpool = ctx.enter_context(tc.tile_pool(name="gp", bufs=6))
    ppool = ctx.enter_context(tc.tile_pool(name="pp", bufs=3, space="PSUM"))

    inv_c = 1.0 / C
    dcount = 0
    n_iter = H // (2 * HB)          # 16 iterations, 2 h-blocks each

    for it in range(n_iter):
        h0 = it * 2 * HB            # slab A rows [h0, h0+8), B [h0+8, h0+16)
        pcs = (2 * it, 2 * it + 1)
        bases = (0, 64)

        xt = xpool.tile([128, HB * W], mybir.dt.bfloat16,
                        name=f"xt{it}", tag="xt")
        yt = ypool.tile([128, LEAD + HB * W], mybir.dt.bfloat16,
                        name=f"yt{it}", tag="yt")

        # casting loads (SWDGE): f32 HBM -> bf16 SBUF
        nc.gpsimd.dma_start(xt[0:C, :], x_ap[:, h0:h0 + HB, :])
        nc.gpsimd.dma_start(xt[64:64 + C, :], x_ap[:, h0 + HB:h0 + 2 * HB, :])
        if it == 0:
            # no rows before row 0: lead cols stay unloaded; the very first
            # w-tile uses a shrunk moving window instead
            nc.gpsimd.dma_start(yt[0:C, LEAD:], yflat[:, 0:HB * W])
        else:
            nc.gpsimd.dma_start(yt[0:C, :],
                                yflat[:, h0 * W - LEAD:(h0 + HB) * W])
        nc.gpsimd.dma_start(yt[64:64 + C, :],
                            yflat[:, (h0 + HB) * W - LEAD:(h0 + 2 * HB) * W])

        gts = [gpool.tile([128, GW8], mybir.dt.float16,
                          name=f"gt{it}_{s}", tag="gt") for s in range(2)]
        for s in range(2):
            # zero the w<d zone (u<47, all hsub); drains never touch it
            nc.vector.memset(gts[s][:, 0:LEAD * HB], 0.0)

        for q in range(HB // 2):         # h-row pairs g = 2q, 2q+1
            ps = {}
            for s in range(2):
                for half in range(NW // 2):
                    ps[s, half] = ppool.tile(
                        [128, 1024], mybir.dt.float32,
                        name=f"ps{it}_{q}_{s}_{half}", tag="ps",
                        padded_shape=[128, 1024])
            for p in range(2):
                g = 2 * q + p
                for half in range(NW // 2):
                    for wsub in range(2):
                        wt = half * 2 + wsub
                        for s in range(2):
                            base = bases[s]
                            lhs = xt[base:base + C,
                                     g * W + wt * 128: g * W + wt * 128 + 128]
                            lo = LEAD if (it == 0 and g == 0 and s == 0
                                          and wt == 0) else 0
                            rhs = yt[base:base + C,
                                     g * W + wt * 128 + lo:
                                     g * W + wt * 128 + MMN]
                            nc.tensor.matmul(
                                ps[s, half][:, 512 * p + wsub * MMN + lo:
                                            512 * p + (wsub + 1) * MMN],
                                lhs, rhs, start=True, stop=True)

            for s in range(2):
                gt = gts[s]
                for half in range(NW // 2):
                    # half 0 skips u<47 (w-tile 0's w<d zone, memset to 0)
                    lo = LEAD if half == 0 else 0
                    gfull = gt[:, :]
                    dst = bass.AP(
                        gfull.tensor,
                        gfull.offset + (half * 2 * MMN + lo) * HB + 2 * q,
                        [[GW8, 128], [HB, 2 * MMN - lo], [1, 2]])
                    pfull = ps[s, half][:, :]
                    src = bass.AP(
                        pfull.tensor, pfull.offset + lo,
                        [[1024, 128], [1, 2 * MMN - lo], [512, 2]])
                    if (dcount % 13) < act_frac * 13:
                        nc.scalar.mul(dst, src, inv_c)
                    else:
                        nc.vector.tensor_scalar_mul(dst, src, inv_c)
                    dcount += 1

        for s in range(2):
            pc = pcs[s]
            # dump the h-block G8 to DRAM scratch (contiguous 1.43 MB)
            dmp = bass.AP(gd, pc * 128 * GW8, [[GW8, 128], [1, GW8]])
            nc.sync.dma_start(dmp, gts[s][:, :])
            # skewed extraction: 768 B diagonal runs -> contiguous output
            src = bass.AP(gd, pc * 128 * GW8,
                          [[GW8 + HB, 128], [MMN * HB, NW], [1, DH]])
            dst = bass.AP(o_t, pc * (128 * NW * DH),
                          [[NW * DH, 128], [DH, NW], [1, DH]])
            nc.sync.dma_start(dst, src)


def _build_kernel():
    nc = bass.Bass(trn_type="TRN2", target_bir_lowering=False)
    x_d = nc.dram_tensor("x", [C, H, W], mybir.dt.float32, kind="ExternalInput")
    y_d = nc.dram_tensor("y", [C, H, W], mybir.dt.float32, kind="ExternalInput")
    o_d = nc.dram_tensor("o", [NBLK, 128, NW, DH], mybir.dt.float16,
                         kind="ExternalOutput")
    with ExitStack() as ctx:
        tc = ctx.enter_context(tile.TileContext(nc))
        _emit_body(ctx, tc, x_d.ap(), y_d.ap(), o_d.ap())
    _split_waits(nc)
    return nc


_NC_CACHE = None


def _get_nc():
    global _NC_CACHE
    if _NC_CACHE is None:
        _NC_CACHE = _build_kernel()
    return _NC_CACHE


def kernel(x: np.ndarray, y: np.ndarray, maxdisp=48) -> np.ndarray:
    assert int(maxdisp) == D
    x = np.ascontiguousarray(np.asarray(x, dtype=np.float32))
    y = np.ascontiguousarray(np.asarray(y, dtype=np.float32))
    assert x.shape == (B, C, H, W) and y.shape == (B, C, H, W)

    nc = _get_nc()
    in_maps = [{"x": x[b], "y": y[b]} for b in range(B)]
    res = bass_utils.run_bass_kernel_spmd(nc, in_maps, core_ids=list(range(B)))

    out = np.empty((B, D, H, W), dtype=np.float32)
    for b in range(B):
        ob = np.asarray(res.results[b]["o"])      # [NBLK, 128, NW, DH] fp16
        arr = ob.reshape(NBLK, 128, NW, D, HB)    # [hblk, j, wt, d_rev, hsub]
        out[b] = (arr[:, :, :, ::-1, :]
                  .transpose(3, 0, 4, 2, 1)       # [d, hblk, hsub, wt, j]
                  .reshape(D, H, W)
                  .astype(np.float32))
    return out


if __name__ == "__main__":
    rng = np.random.default_rng(0)
    x = rng.standard_normal((B, C, H, W), dtype=np.float32)
    y = rng.standard_normal((B, C, H, W), dtype=np.float32)
    out = kernel(x=x, y=y, maxdisp=D)
    print("kernel output:", out.shape, out.dtype)
